# revision 60
# baseline (speedup 1.0000x reference)
"""Trainium2 Bass kernel for the se3ACN encoder (gnn_message_passing).

Strategy (v3: spectral collapse of the radial MLP)
--------------------------------------------------
The per-pair radial MLP (3 -> 150 -> 150 -> 150 -> Cout*Cin, softplus
activations) depends only on the scalar pair distance r, and its weights are
runtime inputs.  On the host we least-squares fit, per cloud, all Cout*Cin
radial output functions in a sine basis evaluated at s = r^2 (the functions
have zero slope in r at 0, so they are smooth in s; using s skips the device
sqrt):

    R_j(s) ~= sum_d A[d, j] * phi_d(s),  phi_d(s) = sin(2pi*(k_d*s/P + c_d))

with harmonics k_d = 0..64 over period P > 9 and phases {0.001, 0.251}
(sin/cos pairs; the small offset keeps device args positive).  Fit residual
at D=128 is ~1e-5 relative -- far below f32r matmul noise.

On device an atom's whole 3-cloud radial evaluation collapses to:
    t = a*w8 + b*u' + c*mask        one K=3 f32r matmul   (k = 8a + b)
    r = (t + 1.5*2^23) - 1.5*2^23   one DVE op: round(t) (no mod in the ISA)
    w = t - r                       one DVE subtract, |2pi*w| <= pi
    phi = sin(2pi * w)              one ACT pass per TWO atoms
where u = s/P, w8 = 8u - round(8u) (computed once in geometry), and w8/u'
are pre-multiplied by the neighbor mask so masked pairs give t = 0 and
phi = sin(0) = 0 exactly -- the cutoff costs nothing.  Integer shifts from
the two-stage harmonics are absorbed by sin periodicity; |t| <= 16 keeps
f32r rounding out of the phase.

Two atoms share the 128 partitions (rows 0:64 even atom, 64:128 odd atom;
DVE/ACT access patterns relocate partition bases freely -- matmul
destinations cannot, they must start at partition 0 on this arch).
phi[d, (m, n)] is shared by all three clouds; each cloud is then one
K=128 einsum matmul per atom PAIR accumulating into one PSUM bank:
acc[o, n] += G_c[:, pair] . phi[:, (pair, n)], with
G_c[d, (o, m)] = sum_i A_c[d, o, i] feat[m, i]/sqrt(cin) (8 small
matmuls + strided even/odd copies).

Sharding: cores (2b, 2b+1) handle molecule b; each owns half the source
atoms m.  Features are AllReduced between clouds 0->1 and 1->2 (a dummy
AllReduce overlapped with phase A pre-warms the CC firmware); the last
cloud's partial features go back to the host, which sums the halves and
runs the lp-pool + tiny 4x24 -> 4x48 batchnorm head.  All f32r matmuls
keep free dim >= 286 (f32r ISA minimum); G-builds are padded.
"""

import math

import numpy as np

import concourse.bass as bass
import concourse.mybir as mybir
import concourse.tile as tile
from concourse import bacc
from concourse.bass_utils import run_bass_kernel_spmd

AF = mybir.ActivationFunctionType
ALU = mybir.AluOpType
F32 = mybir.dt.float32
F32R = mybir.dt.float32r
BF16 = mybir.dt.bfloat16

B, N = 4, 286
EMB, CD, NCLOUD = 4, 8, 3
NCORES = 8

KHARM = 31                   # sin/cos harmonic pairs 1..KHARM (+ k=32 sin)
PERIOD = 9.6                 # sine basis period in s = r^2 units (domain [0, 9])
NGRID = 20001
MAGIC = float(3 * 2 ** 22)   # 1.5*2^23: unit fp32 spacing on both sides
SVG = 8                      # source atoms per staged-row DMA


def _basis_arrays():
    ks = [0]
    ph = [0.251]
    for k in range(1, KHARM + 1):
        ks += [k, k]
        ph += [0.001, 0.251]
    ks += [KHARM + 1]
    ph += [0.001]
    return np.array(ks, np.float64), np.array(ph, np.float64)


_KS, _PH = _basis_arrays()
D = len(_KS)                 # 64 basis functions (two atoms share 128 rows)


def _chunks(total, size=128):
    n = -(-total // size)
    base = total // n
    rem = total % n
    out = []
    off = 0
    for i in range(n):
        pm = base + (1 if i < rem else 0)
        out.append((off, pm))
        off += pm
    return out


class _PackLayout:
    """Column layout of the two packed constant tensors ([128, cols])."""

    def __init__(self, m_own):
        self.m_own = m_own
        # small f32r pack (phase A matmul operands -- loads in ~2us so the
        # per-pair pipeline is not gated on the big G-build weights)
        o = 0
        self.zw = o; o += 128                    # [6, 128] block-diag lhsT:
        # rows 0:3 x cols 0:64 = (a,b,c) for the even atom, rows 3:6 x
        # cols 64:128 the same for the odd atom -- one K=6 matmul fills
        # both partition halves of z
        self.cols_a = o
        # big f32r pack (phase B)
        o = 0
        self.wg = []
        for c in range(NCLOUD):
            self.wg.append(o); o += CD * D       # [cin, CD*D] G-build lhsT
        self.featT0 = o; o += N                  # [EMB, N] padded own-slice feats
        self.cols_r = o
        # float32 pack (geometry + half-select scalars)
        o = 0
        self.geomA = o; o += m_own
        self.geomB = o; o += N
        self.ssel = o; o += 2
        self.cols_f = o


def _build(nc, m_own, use_collective, pdt=F32R):
    """Per-core program: phase A computes phi[d, (m, n)] for its own m's,
    phase B runs the three chained cloud einsums (AllReduce after clouds
    0 and 1; cloud 2 partials are combined on the host)."""
    L = _PackLayout(m_own)

    packa = nc.declare_dram_parameter("packa", [128, L.cols_a], F32R, isOutput=False)
    packr = nc.declare_dram_parameter("packr", [128, L.cols_r], F32R, isOutput=False)
    packf = nc.declare_dram_parameter("packf", [128, L.cols_f], F32, isOutput=False)
    sumsq = nc.declare_dram_parameter("sumsq", [CD, NCLOUD - 1], F32, isOutput=True)
    ft1_dbg = nc.declare_dram_parameter("ft1", [CD, N], F32R, isOutput=True)
    ft2p = nc.declare_dram_parameter("ft2p", [CD, N], F32, isOutput=True)

    groups = [[2 * b, 2 * b + 1] for b in range(NCORES // 2)]
    TWO_PI = 2.0 * math.pi
    n_sv = -(-m_own // SVG)

    with tile.TileContext(nc) as tc:
        with (
            tc.tile_pool(name="const", bufs=1) as cp,
            tc.tile_pool(name="phi", bufs=1) as php,
            tc.tile_pool(name="st", bufs=1) as stp,
            tc.tile_pool(name="sv", bufs=2) as svp,
            tc.tile_pool(name="rt", bufs=3) as hp,
            tc.tile_pool(name="t1", bufs=3) as t1p,
            tc.tile_pool(name="g", bufs=1) as gp,
            tc.tile_pool(name="ft", bufs=1) as ftp,
            tc.tile_pool(name="misc", bufs=1) as mp,
            tc.tile_pool(name="pa", bufs=4, space=bass.MemorySpace.PSUM) as pa,
            tc.tile_pool(name="pb", bufs=2, space=bass.MemorySpace.PSUM) as pb,
            tc.tile_pool(name="pcmb", bufs=2, space=bass.MemorySpace.PSUM) as pcmb,
            tc.tile_pool(name="dstage", bufs=2, space=bass.MemorySpace.DRAM) as dp,
            tc.tile_pool(name="dwrm", bufs=1, space=bass.MemorySpace.DRAM) as dwp,
        ):
            pa_sb = cp.tile([128, L.cols_a], F32R, tag="packa")
            nc.sync.dma_start(out=pa_sb[:], in_=packa[:])
            pf = cp.tile([128, L.cols_f], F32, tag="packf")
            nc.sync.dma_start(out=pf[:], in_=packf[:])

            geomA_sb = pf[0:5, L.geomA:L.geomA + m_own]
            geomB_sb = pf[0:5, L.geomB:L.geomB + N]
            out_sb = cp.tile([CD, NCLOUD - 1], F32, tag="out")

            if use_collective:
                # dummy AllReduce issued first: its ~16us CC firmware
                # kickoff overlaps geometry so the real ones start hot and
                # nothing downstream waits on it (own DRAM pool)
                wrm_in = dwp.tile([CD, 4], F32, tag="wrm_in")
                wrm_out = dwp.tile([CD, 4], F32, tag="wrm_out")
                nc.scalar.dma_start(out=wrm_in[:], in_=pf[0:CD, 0:4])
                nc.gpsimd.collective_compute(
                    "AllReduce", ALU.add, replica_groups=groups,
                    ins=[wrm_in.opt()], outs=[wrm_out.opt()],
                )

            # ---- geometry: s = r^2 -> staged rows (w8, u', mask) where
            # w8 = 8s'/P - round(8s'/P), u' = s'/P, s' = s*mask.  geomA
            # columns are host-reordered even atoms first, so even atoms
            # stage to rows 0:3 and odd atoms to rows 3:6 at the same
            # pair-column -- phase A reads one [6, N] slice per pair.
            npm = (m_own + 1) // 2
            nod = m_own - npm
            stage_d = dp.tile([6, npm * N], F32R, tag="stage_d")
            geo_list = ([(off, pm, 0, off) for off, pm in _chunks(npm)]
                        + [(off, pm, 3, npm + off) for off, pm in _chunks(nod)])
            for (pcol, pm, row0, ga_off) in geo_list:
                s_ps = pa.tile([128, N], F32, tag="pa")
                nc.tensor.matmul(
                    s_ps[0:pm, :], geomA_sb[:, ga_off:ga_off + pm], geomB_sb,
                    start=True, stop=True,
                )
                st = stp.tile([128, 3 * N], F32R, tag="st")
                nc.vector.tensor_scalar(
                    out=st[0:pm, 2 * N:3 * N], in0=s_ps[0:pm, :],
                    scalar1=9.0, scalar2=1.0, op0=ALU.is_lt, op1=ALU.mult,
                )
                spt = mp.tile([128, N], F32, tag="spt")
                nc.vector.tensor_mul(spt[0:pm, :], s_ps[0:pm, :],
                                     st[0:pm, 2 * N:3 * N])
                x8 = mp.tile([128, N], F32, tag="x8")
                nc.vector.tensor_scalar_mul(x8[0:pm, :], spt[0:pm, :],
                                            float(8.0 / PERIOD))
                r8 = mp.tile([128, N], F32, tag="r8")
                nc.vector.tensor_scalar(
                    out=r8[0:pm, :], in0=x8[0:pm, :],
                    scalar1=MAGIC, scalar2=MAGIC, op0=ALU.add, op1=ALU.subtract,
                )
                nc.vector.tensor_sub(st[0:pm, 0:N], x8[0:pm, :], r8[0:pm, :])
                nc.vector.tensor_scalar_mul(st[0:pm, N:2 * N], spt[0:pm, :],
                                            float(1.0 / PERIOD))
                nc.sync.dma_start(
                    out=stage_d[row0:row0 + 3, pcol * N:(pcol + pm) * N]
                        .rearrange("k (p n) -> p k n", p=pm),
                    in_=st[0:pm, :].rearrange("p (k n) -> p k n", k=3),
                )
            if nod < npm:
                # odd tail: zero the duplicate pair's odd-atom rows so its
                # z is 0 -> sin(0) = 0 (its G columns are zeroed as well)
                zt = mp.tile([3, N], F32R, tag="zt")
                nc.vector.tensor_scalar_mul(zt[0:3, :], pf[0:3, L.geomB:
                                                           L.geomB + N], 0.0)
                nc.scalar.dma_start(out=stage_d[3:6, nod * N:npm * N],
                                    in_=zt[0:3, 0:N])
            # big phase-B constants load on the ACT DGE queue so phase A's
            # staged-row loads (sync queue) don't serialize behind the
            # 1.8MB transfer
            pr = cp.tile([128, L.cols_r], F32R, tag="packr")
            nc.scalar.dma_start(out=pr[:], in_=packr[:])

            # ---- phase A: two atoms share the 128 partitions -- rows 0:D
            # hold atom 2p's basis, rows D:128 atom 2p+1's (the last odd
            # pair duplicates its atom; the duplicate column of G is
            # zeroed).  The -I accumulate for pair p issues on the PE after
            # pair p+1's Z matmuls so the in-order PE queue never
            # head-of-line blocks on the DVE round result.
            # Each PAIR of atoms: one K=6 block-diagonal matmul fills both
            # partition halves of z, then one DVE round, one DVE subtract,
            # one Sin pass -- everything amortized over two atoms.
            phi = php.tile([128, npm * N], pdt, tag="phi")
            sv = None
            g0 = 0
            for pi in range(npm):
                if pi % SVG == 0:
                    g0 = pi
                    gsz = min(SVG, npm - g0)
                    sv = svp.tile([6, SVG * N], F32R, tag="sv")
                    nc.sync.dma_start(out=sv[0:6, 0:gsz * N],
                                      in_=stage_d[0:6, g0 * N:(g0 + gsz) * N])
                z_ps = pa.tile([128, N], F32, tag="pa")
                nc.tensor.matmul(z_ps[0:128, :], pa_sb[0:6, L.zw:L.zw + 128],
                                 sv[0:6, (pi - g0) * N:(pi - g0 + 1) * N],
                                 start=True, stop=True)
                rt = hp.tile([128, N], F32R, tag="rt")
                nc.vector.tensor_scalar(
                    out=rt[0:128, :], in0=z_ps[0:128, :],
                    scalar1=MAGIC, scalar2=MAGIC, op0=ALU.add, op1=ALU.subtract,
                )
                t1 = t1p.tile([128, N], F32, tag="t1")
                nc.vector.tensor_sub(t1[0:128, :], z_ps[0:128, :],
                                     rt[0:128, :])
                nc.scalar.activation(
                    phi[0:128, pi * N:(pi + 1) * N],
                    t1[0:128, :], AF.Sin, scale=TWO_PI,
                )

            # ---- phase B: clouds
            featT_prev = pr[0:EMB, L.featT0:L.featT0 + N]   # padded [cin, N]
            for c in range(NCLOUD):
                cin = EMB if c == 0 else CD
                G = gp.tile([128, CD * npm], pdt, tag="G")
                for o in range(CD):
                    g_ps = pb.tile([128, N], F32, tag="pb")
                    nc.tensor.matmul(
                        g_ps[0:D, :],
                        pr[0:cin, L.wg[c] + o * D:L.wg[c] + (o + 1) * D],
                        featT_prev,
                        start=True, stop=True,
                    )
                    nc.scalar.copy(G[0:D, o * npm:(o + 1) * npm],
                                   g_ps[0:D, 0:m_own:2])
                    nc.scalar.copy(G[D:128, o * npm:o * npm + m_own // 2],
                                   g_ps[0:D, 1:m_own:2])
                if m_own % 2 == 1:
                    # duplicate-atom columns: zero their odd-row G entries
                    nc.scalar.mul(G[D:128, npm - 1:CD * npm:npm],
                                  G[0:D, npm - 1:CD * npm:npm], 0.0)

                acc = pcmb.tile([CD, N], F32, tag="acc")
                for p2 in range(npm):
                    nc.tensor.matmul(
                        acc[:], G[0:128, p2:CD * npm:npm],
                        phi[0:128, p2 * N:(p2 + 1) * N],
                        start=(p2 == 0), stop=(p2 == npm - 1),
                    )

                if c == NCLOUD - 1:
                    ft2_sb = ftp.tile([CD, N], F32, tag="ft2")
                    nc.scalar.copy(ft2_sb[:], acc[:])
                    nc.sync.dma_start(out=ft2p[:], in_=ft2_sb[:])
                    break

                ft = ftp.tile([CD, N], F32R, tag="ft")
                if use_collective:
                    ft_part = ftp.tile([CD, N], F32R, tag="ftp")
                    nc.scalar.copy(ft_part[:], acc[:])
                    cc_in = dp.tile([CD, N], F32R, tag="cc_in")
                    cc_out = dp.tile([CD, N], F32R, tag="cc_out")
                    nc.sync.dma_start(out=cc_in[:], in_=ft_part[:])
                    nc.gpsimd.collective_compute(
                        "AllReduce", ALU.add,
                        replica_groups=groups,
                        ins=[cc_in.opt()], outs=[cc_out.opt()],
                    )
                    nc.sync.dma_start(out=ft[:], in_=cc_out[:])
                    # own-m slice selected arithmetically (shared program),
                    # padded to N cols with zeros for the next G-build
                    ft_own = ftp.tile([CD, N], F32R, tag="fto")
                    fo1 = ftp.tile([CD, m_own], F32R, tag="fo1")
                    nc.vector.tensor_scalar_mul(
                        fo1[:], ft[:, 0:m_own],
                        pf[0:CD, L.ssel:L.ssel + 1])
                    fo2 = ftp.tile([CD, m_own], F32R, tag="fo2")
                    nc.vector.tensor_scalar_mul(
                        fo2[:], ft[:, m_own:2 * m_own],
                        pf[0:CD, L.ssel + 1:L.ssel + 2])
                    nc.vector.tensor_add(ft_own[:, 0:m_own], fo1[:], fo2[:])
                    nc.vector.tensor_scalar_mul(
                        ft_own[:, m_own:N], ft[:, m_own:N], 0.0)
                    featT_prev = ft_own[0:CD, 0:N]
                else:
                    nc.scalar.copy(ft[:], acc[:])
                    featT_prev = ft[0:CD, 0:N]
                sq = mp.tile([CD, N], F32, tag="sq")
                nc.scalar.activation(sq[:], ft[:], AF.Square,
                                     accum_out=out_sb[:, c:c + 1])
                if c == 0:
                    nc.sync.dma_start(out=ft1_dbg[:], in_=ft[:])

            nc.sync.dma_start(out=sumsq[:], in_=out_sb[:])
    return nc


_PROG_CACHE = {}
_FIT_CACHE = {}


def _force_act_tables(nc):
    """Pin the ACT table chooser to the single set covering Sin/Square/Copy."""
    import bass_rust as _bass_rust
    from concourse.hw_specs import get_activation_tables

    allowed = {"trig_and_small"}
    tables = [
        (name, (funcs if name in allowed else set()))
        for name, funcs in get_activation_tables(nc.m.arch).items()
    ]

    def _patched():
        has_act = any(
            isinstance(i, mybir.InstActivation)
            for b in nc.main_func.blocks
            for i in b.instructions
        )
        if has_act:
            _bass_rust.insert_act_table_loads(nc, tables)

    nc.insert_act_table_loads = _patched


def _get_program(m_own, use_collective, pdt=F32R):
    key = (m_own, use_collective, pdt)
    if key not in _PROG_CACHE:
        nc = bacc.Bacc(
            "TRN2", target_bir_lowering=False, debug=False,
            num_devices=NCORES,
        )
        _build(nc, m_own, use_collective, pdt)
        _force_act_tables(nc)
        nc.compile()
        _PROG_CACHE[key] = nc
    return _PROG_CACHE[key]


def _f32(x):
    return np.ascontiguousarray(np.asarray(x), dtype=np.float32)


def _fit_radial(rad_W0, rad_W1, rad_W2, rad_Wout0, rad_Wout12):
    """Least-squares fit A_c[d, o*cin+i] of the radial MLP outputs in the
    sine basis over s = r^2 in [0, 9].  Exact float64 MLP evaluation."""
    key = (np.asarray(rad_W0).tobytes(), np.asarray(rad_Wout0).tobytes())
    if key in _FIT_CACHE:
        return _FIT_CACHE[key]
    H = rad_W1.shape[-1]
    s_grid = np.linspace(0.0, 9.0, NGRID)
    r = np.sqrt(s_grid)
    RADII = np.array([0.0, 1.5, 3.0])
    u = (r[:, None] - RADII) / 1.5
    basis = np.where(np.abs(u) < 1.0, np.cos(0.5 * np.pi * u) ** 2, 0.0)

    def spb(x):
        z = 5.0 * x
        return np.where(z > 30, z, np.log1p(np.exp(np.minimum(z, 30)))) / 5.0

    Phi_g = np.sin(2 * np.pi * (_KS[None, :] * s_grid[:, None] / PERIOD
                                + _PH[None, :]))
    wouts = (rad_Wout0, rad_Wout12[0], rad_Wout12[1])
    A_fit = []
    for c in range(NCLOUD):
        x = spb(basis @ np.float64(rad_W0[c]).T / math.sqrt(3.0))
        x = spb(x @ np.float64(rad_W1[c]).T / math.sqrt(H))
        x = spb(x @ np.float64(rad_W2[c]).T / math.sqrt(H))
        R = x @ np.float64(wouts[c]).T / math.sqrt(H)     # [g, CD*cin]
        A, _, _, _ = np.linalg.lstsq(Phi_g, R, rcond=None)
        A_fit.append(A.astype(np.float32))                # [D, CD*cin]
    _FIT_CACHE[key] = A_fit
    return A_fit


def _host_inputs(xyz, Z, emb_W, rad_W0, rad_W1, rad_W2, rad_Wout0, rad_Wout12,
                 m_own, m_starts):
    """Build per-core in_maps: two packed constant tensors per core."""
    L = _PackLayout(m_own)
    xyz = _f32(xyz)
    Z = np.asarray(Z)
    A_fit = _fit_radial(rad_W0, rad_W1, rad_W2, rad_Wout0, rad_Wout12)

    packa_shared = np.zeros((128, L.cols_a), np.float32)
    for h in range(2):
        packa_shared[3 * h + 0, L.zw + D * h:L.zw + D * (h + 1)] = \
            (_KS // 8).astype(np.float32)
        packa_shared[3 * h + 1, L.zw + D * h:L.zw + D * (h + 1)] = \
            (_KS % 8).astype(np.float32)
        packa_shared[3 * h + 2, L.zw + D * h:L.zw + D * (h + 1)] = \
            _PH.astype(np.float32)
    packr_shared = np.zeros((128, L.cols_r), np.float32)
    for c in range(NCLOUD):
        cin = EMB if c == 0 else CD
        # wg[i, o*D+d] = A[d, o*cin+i] / sqrt(cin)
        A = A_fit[c].reshape(D, CD, cin) / np.sqrt(cin).astype(np.float32)
        packr_shared[0:cin, L.wg[c]:L.wg[c] + CD * D] = \
            A.transpose(2, 1, 0).reshape(cin, CD * D)

    emb = _f32(emb_W)
    in_maps = []
    for core in range(NCORES):
        b = core // 2
        x = xyz[b]
        sq = (x * x).sum(-1)
        ones = np.ones(N, np.float32)
        ms = m_starts[core]
        packr = packr_shared.copy()
        packr[0:EMB, L.featT0:L.featT0 + m_own] = \
            emb[Z[b]].T[:, ms:ms + m_own]
        packf = np.zeros((128, L.cols_f), np.float32)
        A2 = np.stack([-2 * x[:, 0], -2 * x[:, 1], -2 * x[:, 2], ones, sq])
        Bm = np.stack([x[:, 0], x[:, 1], x[:, 2], sq, ones])
        own = list(range(ms, ms + m_own))
        order = own[0::2] + own[1::2]            # even atoms first
        packf[0:5, L.geomA:L.geomA + m_own] = A2[:, order]
        packf[0:5, L.geomB:L.geomB + N] = Bm
        packf[0:CD, L.ssel] = 1.0 if ms == 0 else 0.0
        packf[0:CD, L.ssel + 1] = 0.0 if ms == 0 else 1.0
        in_maps.append({"packa": packa_shared, "packr": packr,
                        "packf": packf})
    return in_maps


def run_device(xyz, Z, emb_W, rad_W0, rad_W1, rad_W2, rad_Wout0, rad_Wout12,
               use_collective=True, trace=False, trace_cores=None, rdt=F32R):
    """Run the device part; returns (sumsq [B, 3, CD], BassKernelResults)."""
    m_own = N // 2 if use_collective else N
    m_starts = [(core % 2) * m_own if use_collective else 0
                for core in range(NCORES)]
    pdt = F32R if use_collective else BF16
    nc = _get_program(m_own, use_collective, pdt)
    in_maps = _host_inputs(xyz, Z, emb_W, rad_W0, rad_W1, rad_W2,
                           rad_Wout0, rad_Wout12, m_own, m_starts)
    res = run_bass_kernel_spmd(
        nc, in_maps, list(range(NCORES)), trace=trace,
        trace_cores=trace_cores,
    )
    sumsq = np.zeros((B, NCLOUD, CD), np.float32)
    for b in range(B):
        sumsq[b, 0:2] = res.results[2 * b]["sumsq"].T
        ft2 = res.results[2 * b]["ft2p"]
        if use_collective:
            ft2 = ft2 + res.results[2 * b + 1]["ft2p"]
        sumsq[b, 2] = (ft2 * ft2).sum(axis=1)
    return sumsq, res


def _head(sumsq, W1, b1, g1, be1, W2, b2, g2, be2):
    x = np.sqrt(sumsq.reshape(B, NCLOUD * CD)).astype(np.float32)  # [B, 24]

    def bn(y, g, be):
        m = y.mean(0)
        v = y.var(0)
        return (y - m) / np.sqrt(v + 1e-5) * g + be

    def lrelu(y):
        return np.where(y > 0, y, 0.2 * y).astype(np.float32)

    x = lrelu(bn(x @ _f32(W1).T + _f32(b1), _f32(g1), _f32(be1)))
    x = lrelu(bn(x @ _f32(W2).T + _f32(b2), _f32(g2), _f32(be2)))
    return x.astype(np.float32)


def kernel(xyz, Z, emb_W, rad_W0, rad_W1, rad_W2, rad_Wout0, rad_Wout12,
           W1, b1, g1, be1, W2, b2, g2, be2):
    sumsq, _ = run_device(xyz, Z, emb_W, rad_W0, rad_W1, rad_W2,
                          rad_Wout0, rad_Wout12, use_collective=True)
    return _head(sumsq, W1, b1, g1, be1, W2, b2, g2, be2)


# revision 71
# speedup vs baseline: 1.0825x; 1.0825x over previous
"""Trainium2 Bass kernel for the se3ACN encoder (gnn_message_passing).

Strategy (v3: spectral collapse of the radial MLP)
--------------------------------------------------
The per-pair radial MLP (3 -> 150 -> 150 -> 150 -> Cout*Cin, softplus
activations) depends only on the scalar pair distance r, and its weights are
runtime inputs.  On the host we least-squares fit, per cloud, all Cout*Cin
radial output functions in a sine basis evaluated at s = r^2 (the functions
have zero slope in r at 0, so they are smooth in s; using s skips the device
sqrt):

    R_j(s) ~= sum_d A[d, j] * phi_d(s),  phi_d(s) = sin(2pi*(k_d*s/P + c_d))

with harmonics k_d = 0..64 over period P > 9 and phases {0.001, 0.251}
(sin/cos pairs; the small offset keeps device args positive).  Fit residual
at D=128 is ~1e-5 relative -- far below f32r matmul noise.

On device an atom's whole 3-cloud radial evaluation collapses to:
    t = a*w8 + b*u' + c*mask        one K=3 f32r matmul   (k = 8a + b)
    r = (t + 1.5*2^23) - 1.5*2^23   one DVE op: round(t) (no mod in the ISA)
    w = t - r                       one DVE subtract, |2pi*w| <= pi
    phi = sin(2pi * w)              one ACT pass per TWO atoms
where u = s/P, w8 = 8u - round(8u) (computed once in geometry), and w8/u'
are pre-multiplied by the neighbor mask so masked pairs give t = 0 and
phi = sin(0) = 0 exactly -- the cutoff costs nothing.  Integer shifts from
the two-stage harmonics are absorbed by sin periodicity; |t| <= 16 keeps
f32r rounding out of the phase.

Two atoms share the 128 partitions (rows 0:64 even atom, 64:128 odd atom;
DVE/ACT access patterns relocate partition bases freely -- matmul
destinations cannot, they must start at partition 0 on this arch).
phi[d, (m, n)] is shared by all three clouds; each cloud is then one
K=128 einsum matmul per atom PAIR accumulating into one PSUM bank:
acc[o, n] += G_c[:, pair] . phi[:, (pair, n)], with
G_c[d, (o, m)] = sum_i A_c[d, o, i] feat[m, i]/sqrt(cin) (8 small
matmuls + strided even/odd copies).

Sharding: cores (2b, 2b+1) handle molecule b; each owns half the source
atoms m.  Features are AllReduced between clouds 0->1 and 1->2 (a dummy
AllReduce overlapped with phase A pre-warms the CC firmware); the last
cloud's partial features go back to the host, which sums the halves and
runs the lp-pool + tiny 4x24 -> 4x48 batchnorm head.  All f32r matmuls
keep free dim >= 286 (f32r ISA minimum); G-builds are padded.
"""

import math

import numpy as np

import concourse.bass as bass
import concourse.mybir as mybir
import concourse.tile as tile
from concourse import bacc
from concourse.bass_utils import run_bass_kernel_spmd

AF = mybir.ActivationFunctionType
ALU = mybir.AluOpType
F32 = mybir.dt.float32
F32R = mybir.dt.float32r
BF16 = mybir.dt.bfloat16

B, N = 4, 286
EMB, CD, NCLOUD = 4, 8, 3
NCORES = 8

KHARM = 31                   # sin/cos harmonic pairs 1..KHARM (+ k=32 sin)
PERIOD = 9.6                 # sine basis period in s = r^2 units (domain [0, 9])
NGRID = 20001
MAGIC = float(3 * 2 ** 22)   # 1.5*2^23: unit fp32 spacing on both sides
SVG = 8                      # source atoms per staged-row DMA


def _basis_arrays():
    ks = [0]
    ph = [0.251]
    for k in range(1, KHARM + 1):
        ks += [k, k]
        ph += [0.001, 0.251]
    ks += [KHARM + 1]
    ph += [0.001]
    return np.array(ks, np.float64), np.array(ph, np.float64)


_KS, _PH = _basis_arrays()
D = len(_KS)                 # 64 basis functions (two atoms share 128 rows)


def _chunks(total, size=128):
    n = -(-total // size)
    base = total // n
    rem = total % n
    out = []
    off = 0
    for i in range(n):
        pm = base + (1 if i < rem else 0)
        out.append((off, pm))
        off += pm
    return out


class _PackLayout:
    """Column layout of the two packed constant tensors ([128, cols])."""

    def __init__(self, m_own):
        self.m_own = m_own
        # small f32r pack (phase A matmul operands -- loads in ~2us so the
        # per-pair pipeline is not gated on the big G-build weights)
        o = 0
        self.zw = o; o += 128                    # [6, 128] block-diag lhsT:
        # rows 0:3 x cols 0:64 = (a,b,c) for the even atom, rows 3:6 x
        # cols 64:128 the same for the odd atom -- one K=6 matmul fills
        # both partition halves of z
        self.cols_a = o
        # big f32r pack (phase B)
        o = 0
        self.wg = []
        for c in range(NCLOUD):
            self.wg.append(o); o += CD * D       # [cin, CD*D] G-build lhsT
        self.featT0 = o; o += N                  # [EMB, N] padded own-slice feats
        self.cols_r = o
        # float32 pack (geometry + half-select scalars)
        o = 0
        self.geomA = o; o += m_own
        self.geomB = o; o += N
        self.ssel = o; o += 2
        self.cols_f = o


def _build(nc, m_own, use_collective, pdt=F32R):
    """Per-core program: phase A computes phi[d, (m, n)] for its own m's,
    phase B runs the three chained cloud einsums (AllReduce after clouds
    0 and 1; cloud 2 partials are combined on the host)."""
    L = _PackLayout(m_own)

    packa = nc.declare_dram_parameter("packa", [128, L.cols_a], F32R, isOutput=False)
    packr = nc.declare_dram_parameter("packr", [128, L.cols_r], F32R, isOutput=False)
    packf = nc.declare_dram_parameter("packf", [128, L.cols_f], F32, isOutput=False)
    sumsq = nc.declare_dram_parameter("sumsq", [CD, NCLOUD - 1], F32, isOutput=True)
    ft1_dbg = nc.declare_dram_parameter("ft1", [CD, N], F32R, isOutput=True)
    ft2p = nc.declare_dram_parameter("ft2p", [CD, N], F32, isOutput=True)

    groups = [[2 * b, 2 * b + 1] for b in range(NCORES // 2)]
    TWO_PI = 2.0 * math.pi
    n_sv = -(-m_own // SVG)

    with tile.TileContext(nc) as tc:
        with (
            tc.tile_pool(name="const", bufs=1) as cp,
            tc.tile_pool(name="phi", bufs=1) as php,
            tc.tile_pool(name="st", bufs=1) as stp,
            tc.tile_pool(name="sv", bufs=2) as svp,
            tc.tile_pool(name="rt", bufs=3) as hp,
            tc.tile_pool(name="t1", bufs=3) as t1p,
            tc.tile_pool(name="g", bufs=1) as gp,
            tc.tile_pool(name="ft", bufs=1) as ftp,
            tc.tile_pool(name="misc", bufs=1) as mp,
            tc.tile_pool(name="pa", bufs=4, space=bass.MemorySpace.PSUM) as pa,
            tc.tile_pool(name="pb", bufs=2, space=bass.MemorySpace.PSUM) as pb,
            tc.tile_pool(name="pcmb", bufs=2, space=bass.MemorySpace.PSUM) as pcmb,
            tc.tile_pool(name="dstage", bufs=2, space=bass.MemorySpace.DRAM) as dp,
            tc.tile_pool(name="dwrm", bufs=1, space=bass.MemorySpace.DRAM) as dwp,
        ):
            pa_sb = cp.tile([128, L.cols_a], F32R, tag="packa")
            nc.sync.dma_start(out=pa_sb[:], in_=packa[:])
            pf = cp.tile([128, L.cols_f], F32, tag="packf")
            nc.sync.dma_start(out=pf[:], in_=packf[:])

            geomA_sb = pf[0:5, L.geomA:L.geomA + m_own]
            geomB_sb = pf[0:5, L.geomB:L.geomB + N]
            out_sb = cp.tile([CD, NCLOUD - 1], F32, tag="out")

            if use_collective:
                # dummy AllReduce issued first: its ~16us CC firmware
                # kickoff overlaps geometry so the real ones start hot and
                # nothing downstream waits on it (own DRAM pool)
                wrm_in = dwp.tile([CD, 4], F32, tag="wrm_in")
                wrm_out = dwp.tile([CD, 4], F32, tag="wrm_out")
                nc.sync.dma_start(out=wrm_in[:], in_=pf[0:CD, 0:4])
                nc.gpsimd.collective_compute(
                    "AllReduce", ALU.add, replica_groups=groups,
                    ins=[wrm_in.opt()], outs=[wrm_out.opt()],
                )

            # ---- geometry: s = r^2 -> staged rows (w8, u', mask) where
            # w8 = 8s'/P - round(8s'/P), u' = s'/P, s' = s*mask.  geomA
            # columns are host-reordered even atoms first, so even atoms
            # stage to rows 0:3 and odd atoms to rows 3:6 at the same
            # pair-column -- phase A reads one [6, N] slice per pair.
            npm = (m_own + 1) // 2
            nod = m_own - npm
            stage_d = dp.tile([6, npm * N], F32R, tag="stage_d")
            # four half-chunks, interleaved even/odd, so the first pair
            # group's staged rows (both halves) land early and phase A is
            # not gated on the whole odd-atom write
            geo_e = [(off, pm, 0, off) for off, pm in _chunks(npm, 36)]
            geo_o = [(off, pm, 3, npm + off) for off, pm in _chunks(nod, 36)]
            geo_list = [c for pair in zip(geo_e, geo_o) for c in pair]
            geo_list += geo_e[len(geo_o):] + geo_o[len(geo_e):]
            for (pcol, pm, row0, ga_off) in geo_list:
                s_ps = pa.tile([128, N], F32, tag="pa")
                nc.tensor.matmul(
                    s_ps[0:pm, :], geomA_sb[:, ga_off:ga_off + pm], geomB_sb,
                    start=True, stop=True,
                )
                st = stp.tile([128, 3 * N], F32R, tag="st")
                nc.vector.tensor_scalar(
                    out=st[0:pm, 2 * N:3 * N], in0=s_ps[0:pm, :],
                    scalar1=9.0, scalar2=1.0, op0=ALU.is_lt, op1=ALU.mult,
                )
                spt = mp.tile([128, N], F32, tag="spt")
                nc.vector.tensor_mul(spt[0:pm, :], s_ps[0:pm, :],
                                     st[0:pm, 2 * N:3 * N])
                x8 = mp.tile([128, N], F32, tag="x8")
                nc.vector.tensor_scalar_mul(x8[0:pm, :], spt[0:pm, :],
                                            float(8.0 / PERIOD))
                r8 = mp.tile([128, N], F32, tag="r8")
                nc.vector.tensor_scalar(
                    out=r8[0:pm, :], in0=x8[0:pm, :],
                    scalar1=MAGIC, scalar2=MAGIC, op0=ALU.add, op1=ALU.subtract,
                )
                nc.vector.tensor_sub(st[0:pm, 0:N], x8[0:pm, :], r8[0:pm, :])
                nc.vector.tensor_scalar_mul(st[0:pm, N:2 * N], spt[0:pm, :],
                                            float(1.0 / PERIOD))
                nc.sync.dma_start(
                    out=stage_d[row0:row0 + 3, pcol * N:(pcol + pm) * N]
                        .rearrange("k (p n) -> p k n", p=pm),
                    in_=st[0:pm, :].rearrange("p (k n) -> p k n", k=3),
                )
            if nod < npm:
                # odd tail: zero the duplicate pair's odd-atom rows so its
                # z is 0 -> sin(0) = 0 (its G columns are zeroed as well)
                zt = mp.tile([3, N], F32R, tag="zt")
                nc.vector.tensor_scalar_mul(zt[0:3, :], pf[0:3, L.geomB:
                                                           L.geomB + N], 0.0)
                nc.sync.dma_start(out=stage_d[3:6, nod * N:npm * N],
                                  in_=zt[0:3, 0:N])
            # big phase-B constants load on the ACT DGE queue so phase A's
            # staged-row loads (sync queue) don't serialize behind the
            # 1.8MB transfer
            pr = cp.tile([128, L.cols_r], F32R, tag="packr")
            nc.scalar.dma_start(out=pr[:], in_=packr[:])

            # ---- phase A: two atoms share the 128 partitions -- rows 0:D
            # hold atom 2p's basis, rows D:128 atom 2p+1's (the last odd
            # pair duplicates its atom; the duplicate column of G is
            # zeroed).  The -I accumulate for pair p issues on the PE after
            # pair p+1's Z matmuls so the in-order PE queue never
            # head-of-line blocks on the DVE round result.
            # Each PAIR of atoms: one K=6 block-diagonal matmul fills both
            # partition halves of z, then one DVE round, one DVE subtract,
            # one Sin pass -- everything amortized over two atoms.
            phi = php.tile([128, npm * N], pdt, tag="phi")
            sv = None
            g0 = 0
            for pi in range(npm):
                if pi % SVG == 0:
                    g0 = pi
                    gsz = min(SVG, npm - g0)
                    sv = svp.tile([6, SVG * N], F32R, tag="sv")
                    nc.sync.dma_start(out=sv[0:6, 0:gsz * N],
                                      in_=stage_d[0:6, g0 * N:(g0 + gsz) * N])
                z_ps = pa.tile([128, N], F32, tag="pa")
                nc.tensor.matmul(z_ps[0:128, :], pa_sb[0:6, L.zw:L.zw + 128],
                                 sv[0:6, (pi - g0) * N:(pi - g0 + 1) * N],
                                 start=True, stop=True)
                rt = hp.tile([128, N], F32R, tag="rt")
                nc.vector.tensor_scalar(
                    out=rt[0:128, :], in0=z_ps[0:128, :],
                    scalar1=MAGIC, scalar2=MAGIC, op0=ALU.add, op1=ALU.subtract,
                )
                t1 = t1p.tile([128, N], F32, tag="t1")
                nc.vector.tensor_sub(t1[0:128, :], z_ps[0:128, :],
                                     rt[0:128, :])
                nc.scalar.activation(
                    phi[0:128, pi * N:(pi + 1) * N],
                    t1[0:128, :], AF.Sin, scale=TWO_PI,
                )

            # ---- phase B: clouds
            featT_prev = pr[0:EMB, L.featT0:L.featT0 + N]   # padded [cin, N]
            for c in range(NCLOUD):
                cin = EMB if c == 0 else CD
                G = gp.tile([128, CD * npm], pdt, tag="G")
                for o in range(CD):
                    g_ps = pb.tile([128, N], F32, tag="pb")
                    nc.tensor.matmul(
                        g_ps[0:D, :],
                        pr[0:cin, L.wg[c] + o * D:L.wg[c] + (o + 1) * D],
                        featT_prev,
                        start=True, stop=True,
                    )
                    nc.scalar.copy(G[0:D, o * npm:(o + 1) * npm],
                                   g_ps[0:D, 0:m_own:2])
                    nc.scalar.copy(G[D:128, o * npm:o * npm + m_own // 2],
                                   g_ps[0:D, 1:m_own:2])
                if m_own % 2 == 1:
                    # duplicate-atom columns: zero their odd-row G entries
                    nc.scalar.mul(G[D:128, npm - 1:CD * npm:npm],
                                  G[0:D, npm - 1:CD * npm:npm], 0.0)

                acc = pcmb.tile([CD, N], F32, tag="acc")
                for p2 in range(npm):
                    nc.tensor.matmul(
                        acc[:], G[0:128, p2:CD * npm:npm],
                        phi[0:128, p2 * N:(p2 + 1) * N],
                        start=(p2 == 0), stop=(p2 == npm - 1),
                    )

                if c == NCLOUD - 1:
                    ft2_sb = ftp.tile([CD, N], F32, tag="ft2")
                    nc.scalar.copy(ft2_sb[:], acc[:])
                    nc.sync.dma_start(out=ft2p[:], in_=ft2_sb[:])
                    break

                ft = ftp.tile([CD, N], F32R, tag="ft")
                if use_collective:
                    ft_part = ftp.tile([CD, N], F32R, tag="ftp")
                    nc.scalar.copy(ft_part[:], acc[:])
                    cc_in = dp.tile([CD, N], F32R, tag="cc_in")
                    cc_out = dp.tile([CD, N], F32R, tag="cc_out")
                    nc.sync.dma_start(out=cc_in[:], in_=ft_part[:])
                    nc.gpsimd.collective_compute(
                        "AllReduce", ALU.add,
                        replica_groups=groups,
                        ins=[cc_in.opt()], outs=[cc_out.opt()],
                    )
                    nc.sync.dma_start(out=ft[:], in_=cc_out[:])
                    # own-m slice selected arithmetically (shared program),
                    # padded to N cols with zeros for the next G-build
                    ft_own = ftp.tile([CD, N], F32R, tag="fto")
                    fo1 = ftp.tile([CD, m_own], F32R, tag="fo1")
                    nc.vector.tensor_scalar_mul(
                        fo1[:], ft[:, 0:m_own],
                        pf[0:CD, L.ssel:L.ssel + 1])
                    fo2 = ftp.tile([CD, m_own], F32R, tag="fo2")
                    nc.vector.tensor_scalar_mul(
                        fo2[:], ft[:, m_own:2 * m_own],
                        pf[0:CD, L.ssel + 1:L.ssel + 2])
                    nc.vector.tensor_add(ft_own[:, 0:m_own], fo1[:], fo2[:])
                    nc.vector.tensor_scalar_mul(
                        ft_own[:, m_own:N], ft[:, m_own:N], 0.0)
                    featT_prev = ft_own[0:CD, 0:N]
                else:
                    nc.scalar.copy(ft[:], acc[:])
                    featT_prev = ft[0:CD, 0:N]
                sq = mp.tile([CD, N], F32, tag="sq")
                nc.scalar.activation(sq[:], ft[:], AF.Square,
                                     accum_out=out_sb[:, c:c + 1])
                if c == 0:
                    nc.sync.dma_start(out=ft1_dbg[:], in_=ft[:])

            nc.sync.dma_start(out=sumsq[:], in_=out_sb[:])
    return nc


_PROG_CACHE = {}
_FIT_CACHE = {}


def _force_act_tables(nc):
    """Pin the ACT table chooser to the single set covering Sin/Square/Copy."""
    import bass_rust as _bass_rust
    from concourse.hw_specs import get_activation_tables

    allowed = {"trig_and_small"}
    tables = [
        (name, (funcs if name in allowed else set()))
        for name, funcs in get_activation_tables(nc.m.arch).items()
    ]

    def _patched():
        has_act = any(
            isinstance(i, mybir.InstActivation)
            for b in nc.main_func.blocks
            for i in b.instructions
        )
        if has_act:
            _bass_rust.insert_act_table_loads(nc, tables)

    nc.insert_act_table_loads = _patched


def _get_program(m_own, use_collective, pdt=F32R):
    key = (m_own, use_collective, pdt)
    if key not in _PROG_CACHE:
        nc = bacc.Bacc(
            "TRN2", target_bir_lowering=False, debug=False,
            num_devices=NCORES,
        )
        _build(nc, m_own, use_collective, pdt)
        _force_act_tables(nc)
        nc.compile()
        _PROG_CACHE[key] = nc
    return _PROG_CACHE[key]


def _f32(x):
    return np.ascontiguousarray(np.asarray(x), dtype=np.float32)


def _fit_radial(rad_W0, rad_W1, rad_W2, rad_Wout0, rad_Wout12):
    """Least-squares fit A_c[d, o*cin+i] of the radial MLP outputs in the
    sine basis over s = r^2 in [0, 9].  Exact float64 MLP evaluation."""
    key = (np.asarray(rad_W0).tobytes(), np.asarray(rad_Wout0).tobytes())
    if key in _FIT_CACHE:
        return _FIT_CACHE[key]
    H = rad_W1.shape[-1]
    s_grid = np.linspace(0.0, 9.0, NGRID)
    r = np.sqrt(s_grid)
    RADII = np.array([0.0, 1.5, 3.0])
    u = (r[:, None] - RADII) / 1.5
    basis = np.where(np.abs(u) < 1.0, np.cos(0.5 * np.pi * u) ** 2, 0.0)

    def spb(x):
        z = 5.0 * x
        return np.where(z > 30, z, np.log1p(np.exp(np.minimum(z, 30)))) / 5.0

    Phi_g = np.sin(2 * np.pi * (_KS[None, :] * s_grid[:, None] / PERIOD
                                + _PH[None, :]))
    wouts = (rad_Wout0, rad_Wout12[0], rad_Wout12[1])
    A_fit = []
    for c in range(NCLOUD):
        x = spb(basis @ np.float64(rad_W0[c]).T / math.sqrt(3.0))
        x = spb(x @ np.float64(rad_W1[c]).T / math.sqrt(H))
        x = spb(x @ np.float64(rad_W2[c]).T / math.sqrt(H))
        R = x @ np.float64(wouts[c]).T / math.sqrt(H)     # [g, CD*cin]
        A, _, _, _ = np.linalg.lstsq(Phi_g, R, rcond=None)
        A_fit.append(A.astype(np.float32))                # [D, CD*cin]
    _FIT_CACHE[key] = A_fit
    return A_fit


def _host_inputs(xyz, Z, emb_W, rad_W0, rad_W1, rad_W2, rad_Wout0, rad_Wout12,
                 m_own, m_starts):
    """Build per-core in_maps: two packed constant tensors per core."""
    L = _PackLayout(m_own)
    xyz = _f32(xyz)
    Z = np.asarray(Z)
    A_fit = _fit_radial(rad_W0, rad_W1, rad_W2, rad_Wout0, rad_Wout12)

    packa_shared = np.zeros((128, L.cols_a), np.float32)
    for h in range(2):
        packa_shared[3 * h + 0, L.zw + D * h:L.zw + D * (h + 1)] = \
            (_KS // 8).astype(np.float32)
        packa_shared[3 * h + 1, L.zw + D * h:L.zw + D * (h + 1)] = \
            (_KS % 8).astype(np.float32)
        packa_shared[3 * h + 2, L.zw + D * h:L.zw + D * (h + 1)] = \
            _PH.astype(np.float32)
    packr_shared = np.zeros((128, L.cols_r), np.float32)
    for c in range(NCLOUD):
        cin = EMB if c == 0 else CD
        # wg[i, o*D+d] = A[d, o*cin+i] / sqrt(cin)
        A = A_fit[c].reshape(D, CD, cin) / np.sqrt(cin).astype(np.float32)
        packr_shared[0:cin, L.wg[c]:L.wg[c] + CD * D] = \
            A.transpose(2, 1, 0).reshape(cin, CD * D)

    emb = _f32(emb_W)
    in_maps = []
    for core in range(NCORES):
        b = core // 2
        x = xyz[b]
        sq = (x * x).sum(-1)
        ones = np.ones(N, np.float32)
        ms = m_starts[core]
        packr = packr_shared.copy()
        packr[0:EMB, L.featT0:L.featT0 + m_own] = \
            emb[Z[b]].T[:, ms:ms + m_own]
        packf = np.zeros((128, L.cols_f), np.float32)
        A2 = np.stack([-2 * x[:, 0], -2 * x[:, 1], -2 * x[:, 2], ones, sq])
        Bm = np.stack([x[:, 0], x[:, 1], x[:, 2], sq, ones])
        own = list(range(ms, ms + m_own))
        order = own[0::2] + own[1::2]            # even atoms first
        packf[0:5, L.geomA:L.geomA + m_own] = A2[:, order]
        packf[0:5, L.geomB:L.geomB + N] = Bm
        packf[0:CD, L.ssel] = 1.0 if ms == 0 else 0.0
        packf[0:CD, L.ssel + 1] = 0.0 if ms == 0 else 1.0
        in_maps.append({"packa": packa_shared, "packr": packr,
                        "packf": packf})
    return in_maps


def run_device(xyz, Z, emb_W, rad_W0, rad_W1, rad_W2, rad_Wout0, rad_Wout12,
               use_collective=True, trace=False, trace_cores=None, rdt=F32R):
    """Run the device part; returns (sumsq [B, 3, CD], BassKernelResults)."""
    m_own = N // 2 if use_collective else N
    m_starts = [(core % 2) * m_own if use_collective else 0
                for core in range(NCORES)]
    pdt = F32R if use_collective else BF16
    nc = _get_program(m_own, use_collective, pdt)
    in_maps = _host_inputs(xyz, Z, emb_W, rad_W0, rad_W1, rad_W2,
                           rad_Wout0, rad_Wout12, m_own, m_starts)
    res = run_bass_kernel_spmd(
        nc, in_maps, list(range(NCORES)), trace=trace,
        trace_cores=trace_cores,
    )
    sumsq = np.zeros((B, NCLOUD, CD), np.float32)
    for b in range(B):
        sumsq[b, 0:2] = res.results[2 * b]["sumsq"].T
        ft2 = res.results[2 * b]["ft2p"]
        if use_collective:
            ft2 = ft2 + res.results[2 * b + 1]["ft2p"]
        sumsq[b, 2] = (ft2 * ft2).sum(axis=1)
    return sumsq, res


def _head(sumsq, W1, b1, g1, be1, W2, b2, g2, be2):
    x = np.sqrt(sumsq.reshape(B, NCLOUD * CD)).astype(np.float32)  # [B, 24]

    def bn(y, g, be):
        m = y.mean(0)
        v = y.var(0)
        return (y - m) / np.sqrt(v + 1e-5) * g + be

    def lrelu(y):
        return np.where(y > 0, y, 0.2 * y).astype(np.float32)

    x = lrelu(bn(x @ _f32(W1).T + _f32(b1), _f32(g1), _f32(be1)))
    x = lrelu(bn(x @ _f32(W2).T + _f32(b2), _f32(g2), _f32(be2)))
    return x.astype(np.float32)


def kernel(xyz, Z, emb_W, rad_W0, rad_W1, rad_W2, rad_Wout0, rad_Wout12,
           W1, b1, g1, be1, W2, b2, g2, be2):
    sumsq, _ = run_device(xyz, Z, emb_W, rad_W0, rad_W1, rad_W2,
                          rad_Wout0, rad_Wout12, use_collective=True)
    return _head(sumsq, W1, b1, g1, be1, W2, b2, g2, be2)
